# revision 27
# baseline (speedup 1.0000x reference)
"""AdvancedMuonAttention on 8 TRN2 NeuronCores (Bass/Tile SPMD).

Sharding: 8 cores = 4 pairs. Pair p owns (batch=p//2, query tokens
(p%2)*1024..+1024). Within a pair, core A takes heads 0-7, core B heads
8-15 (tensor-parallel on the head dim: wq/wk/wv column-sliced, wo
row-sliced). Each core computes k/v projections for its batch+heads over
all 2048 keys, attention for its 8 heads x 1024 queries, a partial
output projection, then a 2-rank ReduceScatter(add) with its pair
partner resolves the head-dim partial sums and hands each core 512
tokens for RMSNorm + output.

Math notes vs the reference:
  - softmax is computed without max-subtraction (scores are bounded by
    dk/sqrt(dk)/temp since the feature maps are tanh outputs), with the
    0/1 mask applied multiplicatively AFTER exp (identical result).
  - row sums for the softmax denominator come from an appended
    ones-column on V (ctx' = P @ [V|1]), normalization happens on ctx.
  - bv/bo fold into a constant row c* = bv @ wo + bo added to the
    partial outputs before the ReduceScatter; temperature folds into
    the exp() input scale.

Schedule notes: engines execute their instruction streams in-order, so
feature/ctx matmuls are emitted lagged behind the instructions that
produce their inputs (software pipelining) to keep TensorE dense and
the HAM clock warm.
"""

import os
import sys
import numpy as np
from contextlib import ExitStack

for _p in ("/opt/trn_rl_repo",):
    if _p not in sys.path and os.path.isdir(_p):
        sys.path.append(_p)

import concourse.bass as bass
import concourse.bacc as bacc
import concourse.mybir as mybir
import concourse.tile as tile
from concourse.bass_utils import run_bass_kernel_spmd

F32 = mybir.dt.float32
F32R = mybir.dt.float32r
F16 = mybir.dt.float16
AF = mybir.ActivationFunctionType
OP = mybir.AluOpType

B, S, D, H, DK = 2, 2048, 1024, 16, 64
N_CORES = 8
EPS = 1e-6
QTOK = 1024          # query tokens per pair (and per core)
HE = 512             # head dims per core (8 heads x 64)
CTX_LAG = 4          # attention: ctx matmuls lag scores by this many k-tiles


def build_program(debug_taps=False):
    nc = bacc.Bacc("TRN2", debug=False, num_devices=N_CORES)

    qt = nc.declare_dram_parameter("qt", [D, QTOK], F32R, isOutput=False)
    kt = nc.declare_dram_parameter("kt", [D, S], F32R, isOutput=False)
    vt = nc.declare_dram_parameter("vt", [D, S], F32R, isOutput=False)
    maskt = nc.declare_dram_parameter("maskt", [S, QTOK], F16, isOutput=False)
    wq = nc.declare_dram_parameter("wq", [D, HE], F32R, isOutput=False)
    wk = nc.declare_dram_parameter("wk", [D, HE], F32R, isOutput=False)
    wv = nc.declare_dram_parameter("wv", [D, HE], F32R, isOutput=False)
    wo = nc.declare_dram_parameter("wo", [HE, D], F32R, isOutput=False)
    bq = nc.declare_dram_parameter("bq", [4, 128], F32, isOutput=False)
    bk = nc.declare_dram_parameter("bk", [4, 128], F32, isOutput=False)
    bv = nc.declare_dram_parameter("bv", [4, 128], F32R, isOutput=False)
    bo = nc.declare_dram_parameter("bo", [1, D], F32, isOutput=False)
    wnq = nc.declare_dram_parameter("wnq", [DK, DK], F32R, isOutput=False)
    wnk = nc.declare_dram_parameter("wnk", [DK, DK], F32R, isOutput=False)
    bnq = nc.declare_dram_parameter("bnq", [DK, 1], F32, isOutput=False)
    bnk = nc.declare_dram_parameter("bnk", [DK, 1], F32, isOutput=False)
    temp = nc.declare_dram_parameter("temp", [1, 8], F32, isOutput=False)
    gamma = nc.declare_dram_parameter("gamma", [1, D], F32, isOutput=False)
    out_ext = nc.declare_dram_parameter("out", [QTOK, D], F32, isOutput=True)

    # collective bounce buffers
    rs_in = nc.dram_tensor("rs_in", [QTOK, D], F16)
    rs_out = nc.dram_tensor("rs_out", [2 * QTOK, D], F16)
    dbg = {}
    if debug_taps:
        dbg["kfn"] = nc.declare_dram_parameter("dbg_kfn", [128, 4 * S], F16, isOutput=True)
        dbg["qfn"] = nc.declare_dram_parameter("dbg_qfn", [128, 4 * QTOK], F16, isOutput=True)
        dbg["vsb"] = nc.declare_dram_parameter("dbg_vsb", [128, 16 * 520], F16, isOutput=True)
        dbg["ctx"] = nc.declare_dram_parameter("dbg_ctx", [128, 4 * QTOK], F32, isOutput=True)
        dbg["p00"] = nc.declare_dram_parameter("dbg_p00", [128, QTOK], F16, isOutput=True)
        dbg["rsin"] = nc.declare_dram_parameter("dbg_rsin", [QTOK, D], F32, isOutput=True)
        dbg["rsout"] = nc.declare_dram_parameter("dbg_rsout", [512, D], F32, isOutput=True)

    with TileCtx(nc) as tc, ExitStack() as top:
        persist = top.enter_context(tc.tile_pool(name="persist", bufs=1))

        # ---- persistent tiles -------------------------------------------
        mask_sb = persist.tile([128, 16 * QTOK], F16, tag="mask")
        kfn_sb = persist.tile([128, 4 * S], F16, tag="kfn")      # [he-pair rows, pair-block of 2048 tok]
        qfn_sb = persist.tile([128, 4 * QTOK], F16, tag="qfn")
        v_sb = persist.tile([128, 16 * 520], F16, tag="vsb")     # per tok-tile: 8 heads x (64 v + 1 one)
        ctx_sb = persist.tile([128, 4 * QTOK], F32R, tag="ctx")
        cstar = persist.tile([1, D], F32, tag="cstar")
        gamma_sb = persist.tile([1, D], F32, tag="gamma")
        bo_sb = persist.tile([1, D], F32, tag="bo")
        bq_sb = persist.tile([128, 4], F32, tag="bq")
        bk_sb = persist.tile([128, 4], F32, tag="bk")
        bv_sb = persist.tile([128, 4], F32R, tag="bv")
        wnq_d = persist.tile([128, 2 * DK], F32R, tag="wnq")   # block-diag [[wn,0],[0,wn]]
        wnk_d = persist.tile([128, 2 * DK], F32R, tag="wnk")
        bnq_d = persist.tile([128, 1], F32, tag="bnq")
        bnk_d = persist.tile([128, 1], F32, tag="bnk")
        eps_t = persist.tile([128, 1], F32, tag="eps")
        ts_raw = persist.tile([1, 8], F32, tag="tsraw")
        ts_rec = persist.tile([1, 8], F32, tag="tsrec")
        s_bc = persist.tile([128, 8], F32, tag="sbc")
        gamma_bc = persist.tile([128, D], F32, tag="gammabc")
        cstar_bc = persist.tile([128, D], F32, tag="cstarbc")

        # ---- constants / small DMAs -------------------------------------
        nc.vector.memset(eps_t[:], EPS)
        nc.sync.dma_start(gamma_sb[:], gamma[:, :])
        nc.sync.dma_start(bo_sb[:], bo[:, :])
        nc.sync.dma_start(bq_sb[:], bq.ap().rearrange("t p -> p t"))
        nc.sync.dma_start(bk_sb[:], bk.ap().rearrange("t p -> p t"))
        nc.sync.dma_start(bv_sb[:], bv.ap().rearrange("t p -> p t"))
        nc.vector.memset(wnq_d[:].bitcast(F32), 0.0)
        nc.vector.memset(wnk_d[:].bitcast(F32), 0.0)
        nc.sync.dma_start(wnq_d[0:64, 0:64], wnq[:, :])
        nc.sync.dma_start(wnq_d[64:128, 64:128], wnq[:, :])
        nc.sync.dma_start(wnk_d[0:64, 0:64], wnk[:, :])
        nc.sync.dma_start(wnk_d[64:128, 64:128], wnk[:, :])
        nc.sync.dma_start(bnq_d[0:64, :], bnq[:, :])
        nc.sync.dma_start(bnq_d[64:128, :], bnq[:, :])
        nc.sync.dma_start(bnk_d[0:64, :], bnk[:, :])
        nc.sync.dma_start(bnk_d[64:128, :], bnk[:, :])
        nc.sync.dma_start(ts_raw[:], temp[:, :])
        # exp input scale per head: 1/(sqrt(DK) * temperature_h)
        nc.vector.reciprocal(ts_rec[:], ts_raw[:])
        nc.vector.tensor_scalar_mul(ts_rec[:], ts_rec[:], 1.0 / np.sqrt(DK))
        nc.gpsimd.partition_broadcast(s_bc[:], ts_rec[0:1, :])
        nc.gpsimd.partition_broadcast(gamma_bc[:], gamma_sb[0:1, :])

        # ones columns for the V|1 trick
        nc.gpsimd.memset(v_sb[:], 1.0)

        # ==== Phase 1: projections + feature maps ========================
        with ExitStack() as ph1:
            wpool = ph1.enter_context(tc.tile_pool(name="wpool", bufs=1))
            inpool = ph1.enter_context(tc.tile_pool(name="inpool", bufs=10))
            vtpool = ph1.enter_context(tc.tile_pool(name="vtpool", bufs=8))
            kqpool = ph1.enter_context(tc.tile_pool(name="kqpool", bufs=6))
            ppsum = ph1.enter_context(tc.tile_pool(name="ppsum", bufs=3, space="PSUM"))
            fpsum = ph1.enter_context(tc.tile_pool(name="fpsum", bufs=2, space="PSUM"))
            wq_sb = wpool.tile([128, 8 * HE], F32R, tag="wq")
            wk_sb = wpool.tile([128, 8 * HE], F32R, tag="wk")
            wv_sb = wpool.tile([128, 8 * HE], F32R, tag="wv")
            # interleave the first k-stream chunk with wk so the first
            # projection chain starts as early as possible
            kt_j0 = []
            for dt_ in range(8):
                nc.sync.dma_start(wk_sb[:, dt_ * HE:(dt_ + 1) * HE], wk[dt_ * 128:(dt_ + 1) * 128, :])
                t = inpool.tile([128, 512], F32R, tag="instream", name=f"kt0_{dt_}")
                nc.scalar.dma_start(t[:], kt[dt_ * 128:(dt_ + 1) * 128, 0:512])
                kt_j0.append(t)


            # software pipeline: the feature matmul for step i is emitted
            # during step i+1, so TensorE never waits on the DVE bias-add.
            feat_pend = []

            def feat_flush_one():
                xsb, wn_d, bn_d, dst = feat_pend.pop(0)
                fps = fpsum.tile([128, 512], F32, tag="feat")
                nc.tensor.matmul(fps[:], lhsT=wn_d[:], rhs=xsb[:],
                                 start=True, stop=True)
                nc.scalar.activation(dst, fps[:], AF.Tanh, bias=bn_d[:, 0:1])

            def proj_feat(src_dram, w_sb, bias_sb, wn_d, bn_d, fn_sb, n_tok, pre0=None):
                """project src (transposed [D, n_tok]) through w (column
                slice) then per-head feature map tanh((x+b) @ wn + bn),
                writing fp16 features [he, n_tok] into fn_sb."""
                for j in range(n_tok // 512):
                    if j == 0 and pre0 is not None:
                        ins = pre0
                    else:
                        ins = []
                        for dt_ in range(8):
                            t = inpool.tile([128, 512], F32R, tag="instream")
                            nc.scalar.dma_start(t[:], src_dram[dt_ * 128:(dt_ + 1) * 128, j * 512:(j + 1) * 512])
                            ins.append(t)
                    for i in range(4):
                        ps = ppsum.tile([128, 512], F32, tag="proj")
                        for dt_ in range(8):
                            nc.tensor.matmul(
                                ps[:],
                                lhsT=w_sb[:, dt_ * HE + i * 128: dt_ * HE + (i + 1) * 128],
                                rhs=ins[dt_][:],
                                start=(dt_ == 0), stop=(dt_ == 7),
                            )
                        xsb = kqpool.tile([128, 512], F32R, tag="xsb")
                        nc.vector.tensor_scalar_add(xsb[:], ps[:], bias_sb[:, i:i + 1])
                        feat_pend.append((
                            xsb, wn_d, bn_d,
                            fn_sb[:, i * n_tok + j * 512: i * n_tok + (j + 1) * 512],
                        ))
                        if len(feat_pend) > 1:
                            feat_flush_one()

            for dt_ in range(8):
                nc.sync.dma_start(wq_sb[:, dt_ * HE:(dt_ + 1) * HE], wq[dt_ * 128:(dt_ + 1) * 128, :])
            proj_feat(kt, wk_sb, bk_sb, wnk_d, bnk_d, kfn_sb, S, pre0=kt_j0)
            for dt_ in range(8):
                nc.sync.dma_start(wv_sb[:, dt_ * HE:(dt_ + 1) * HE], wv[dt_ * 128:(dt_ + 1) * 128, :])
            proj_feat(qt, wq_sb, bq_sb, wnq_d, bnq_d, qfn_sb, QTOK)
            while feat_pend:
                feat_flush_one()
            # fold the per-head softmax scale 1/(sqrt(dk)*temp_h) into qfn
            for h in range(8):
                pr, off = h // 2, (h % 2) * 64
                sl = qfn_sb[off:off + 64, pr * QTOK:(pr + 1) * QTOK]
                nc.vector.tensor_scalar_mul(sl, sl, s_bc[0:64, h:h + 1])

            # v projection: [tok, he] via VT-tiles as stationary.
            # The v_sb copy lags one chain so TensorE stays dense.
            def v_copy(vps0, t0):
                dst = v_sb[:, t0 * 520:(t0 + 1) * 520].rearrange("p (h c) -> p h c", c=65)[:, :, 0:64]
                nc.vector.tensor_copy(dst, vps0[:].rearrange("p (h c) -> p h c", c=64))

            v_pend = []
            for tc4 in range(4):
                vts = []
                for dt_ in range(8):
                    vtt = vtpool.tile([128, 512], F32R, tag="vtstream")
                    nc.scalar.dma_start(vtt[:], vt[dt_ * 128:(dt_ + 1) * 128, tc4 * 512:(tc4 + 1) * 512])
                    vts.append(vtt)
                for sub in range(4):
                    t = tc4 * 4 + sub
                    vps = ppsum.tile([128, 512], F32, tag="proj")
                    for dt_ in range(8):
                        nc.tensor.matmul(
                            vps[:], lhsT=vts[dt_][:, sub * 128:(sub + 1) * 128],
                            rhs=wv_sb[:, dt_ * HE:(dt_ + 1) * HE],
                            start=(dt_ == 0), stop=(dt_ == 7),
                        )
                    v_pend.append((vps, t))
                    if len(v_pend) > 1:
                        v_copy(*v_pend.pop(0))
            while v_pend:
                v_copy(*v_pend.pop(0))

        # wo is only consumed in phase 3, but its DMA is issued here so it
        # overlaps the attention phase
        wopool = top.enter_context(tc.tile_pool(name="wopool", bufs=1))
        wo_sb = wopool.tile([128, 4 * D], F32R, tag="wo")
        for pr in range(4):
            nc.sync.dma_start(
                wo_sb[:, pr * D:(pr + 1) * D], wo[pr * 128:(pr + 1) * 128, :]
            )

        # ==== Phase 2: attention =========================================
        with ExitStack() as ph2:
            spsum = ph2.enter_context(tc.tile_pool(name="spsum", bufs=2, space="PSUM"))
            cpsum = ph2.enter_context(tc.tile_pool(name="cpsum", bufs=2, space="PSUM"))
            ptpool = ph2.enter_context(tc.tile_pool(name="ptpool", bufs=10))
            rpool = ph2.enter_context(tc.tile_pool(name="rpool", bufs=2))

            # mask tiles: k-tile kti lives at cols kti*QTOK (deferred DMA so
            # the startup path belongs to the projection streams)
            for kti in range(16):
                nc.sync.dma_start(
                    mask_sb[:, kti * QTOK:(kti + 1) * QTOK],
                    maskt[kti * 128:(kti + 1) * 128, :],
                )

            # One flat software pipeline over (head, k-tile): the ctx
            # accumulation lags the scores stream by CTX_LAG steps and the
            # per-head normalize is emitted inline, so TensorE crosses head
            # boundaries without draining.
            steps = [(h, kti) for h in range(8) for kti in range(16)]
            cps_by_head = {}
            pts = {}

            def ctx_step(h, kti):
                cps2 = cps_by_head[h]
                pt = pts.pop((h, kti))
                for qh in range(2):
                    nc.tensor.matmul(
                        cps2[:, qh * 512:(qh + 1) * 512],
                        lhsT=v_sb[:, kti * 520 + h * 65: kti * 520 + (h + 1) * 65],
                        rhs=pt[:, qh * 512:(qh + 1) * 512],
                        start=(kti == 0), stop=(kti == 15),
                    )
                if kti == 15:
                    pr, off = h // 2, (h % 2) * 64
                    rec = rpool.tile([1, QTOK], F32, tag="rec")
                    nc.vector.reciprocal(rec[:], cps2[64:65, :])
                    rec_bc = rpool.tile([64, QTOK], F32, tag="recbc")
                    nc.gpsimd.partition_broadcast(rec_bc[:], rec[0:1, :])
                    nc.vector.tensor_tensor(
                        ctx_sb[off:off + 64, pr * QTOK:(pr + 1) * QTOK],
                        cps2[0:64, :], rec_bc[:], OP.mult,
                    )
                    del cps_by_head[h]

            for idx, (h, kti) in enumerate(steps):
                pr, off = h // 2, (h % 2) * 64
                if kti == 0:
                    cps_by_head[h] = cpsum.tile([65, QTOK], F32, tag="ctxps", name=f"ctxps{h}")
                if idx >= CTX_LAG:
                    ctx_step(*steps[idx - CTX_LAG])
                sps = spsum.tile([128, QTOK], F32, tag="scores")
                for qh in range(2):
                    nc.tensor.matmul(
                        sps[:, qh * 512:(qh + 1) * 512],
                        lhsT=kfn_sb[off:off + 64, pr * S + kti * 128: pr * S + (kti + 1) * 128],
                        rhs=qfn_sb[off:off + 64, pr * QTOK + qh * 512: pr * QTOK + (qh + 1) * 512],
                        start=True, stop=True,
                    )
                pt = ptpool.tile([128, QTOK], F16, tag="pt")
                nc.scalar.activation(pt[:], sps[:], AF.Exp)
                nc.vector.tensor_mul(pt[:], pt[:], mask_sb[:, kti * QTOK:(kti + 1) * QTOK])
                if debug_taps and h == 0 and kti == 0:
                    nc.sync.dma_start(dbg["p00"][:, :], pt[:])
                pts[(h, kti)] = pt
            for idx in range(len(steps) - CTX_LAG, len(steps)):
                ctx_step(*steps[idx])

        if debug_taps:
            nc.sync.dma_start(dbg["kfn"][:, :], kfn_sb[:])
            nc.sync.dma_start(dbg["qfn"][:, :], qfn_sb[:])
            nc.sync.dma_start(dbg["vsb"][:, :], v_sb[:])
            nc.sync.dma_start(dbg["ctx"][:, :], ctx_sb[:].bitcast(F32))

        # ==== Phase 3: output projection + chunked ReduceScatter =========
        # Two RS chunks: chunk c covers pair-token rows c*512..(c+1)*512.
        # The even core ends up with rows [c*512, c*512+256) of each chunk.
        with ExitStack() as ph3:
            opsum = ph3.enter_context(tc.tile_pool(name="opsum", bufs=2, space="PSUM"))
            obpool = ph3.enter_context(tc.tile_pool(name="obpool", bufs=3))
            fpool = ph3.enter_context(tc.tile_pool(name="fpool", bufs=3))
            sqpool = ph3.enter_context(tc.tile_pool(name="sqpool", bufs=2))
            # c* = bv @ wo + 0.5*bo   (both pair cores add 0.5*bo)
            cps = opsum.tile([1, D], F32, tag="cstar_ps")
            for pr in range(4):
                for nh in range(2):
                    nc.tensor.matmul(
                        cps[:, nh * 512:(nh + 1) * 512],
                        lhsT=bv_sb[:, pr:pr + 1],
                        rhs=wo_sb[:, pr * D + nh * 512: pr * D + (nh + 1) * 512],
                        start=(pr == 0), stop=(pr == 3),
                    )
            nc.vector.tensor_scalar_mul(cstar[:], bo_sb[:], 0.5)
            nc.vector.tensor_add(cstar[:], cstar[:], cps[:])
            nc.gpsimd.partition_broadcast(cstar_bc[:], cstar[0:1, :])

            def ob_flush(ops0, t0):
                ob = obpool.tile([128, D], F16, tag="ob")
                nc.vector.tensor_tensor(ob[:], ops0[:], cstar_bc[:], OP.add)
                nc.sync.dma_start(rs_in[t0 * 128:(t0 + 1) * 128, :], ob[:])

            def rmsnorm_chunk(c):
                # ag out: rows [0:1024) = rank-0 partials, [1024:2048) =
                # rank-1 partials, both in pair-token order. Each core norms
                # all 1024 pair tokens (SPMD-uniform); the host keeps the
                # 512 rows this core owns.
                if True:
                    r0 = c * 128
                    oa = fpool.tile([128, D], F16, tag="oa")
                    nc.sync.dma_start(oa[:], rs_out[r0:r0 + 128, :])
                    obp = fpool.tile([128, D], F16, tag="obp")
                    nc.sync.dma_start(obp[:], rs_out[QTOK + r0:QTOK + r0 + 128, :])
                    o = fpool.tile([128, D], F32, tag="o")
                    nc.vector.tensor_add(o[:], oa[:], obp[:])
                    sq = sqpool.tile([128, D], F16, tag="sq")
                    ss = sqpool.tile([128, 1], F32, tag="ss")
                    nc.scalar.activation(sq[:], o[:], AF.Square, accum_out=ss[:])
                    rms = sqpool.tile([128, 1], F32, tag="rms")
                    nc.scalar.activation(rms[:], ss[:], AF.Sqrt, bias=eps_t[:, 0:1], scale=1.0 / D)
                    rinv = sqpool.tile([128, 1], F32, tag="rinv")
                    nc.vector.reciprocal(rinv[:], rms[:])
                    o2 = fpool.tile([128, D], F32, tag="o2")
                    nc.vector.tensor_scalar_mul(o2[:], o[:], rinv[:, 0:1])
                    nc.gpsimd.tensor_tensor(o2[:], o2[:], gamma_bc[:], OP.mult)
                    nc.sync.dma_start(out_ext[r0:r0 + 128, :], o2[:])

            ob_pend = []
            for t in range(8):
                ops = opsum.tile([128, D], F32, tag="ops")
                for pr in range(4):
                    for nh in range(2):
                        nc.tensor.matmul(
                            ops[:, nh * 512:(nh + 1) * 512],
                            lhsT=ctx_sb[:, pr * QTOK + t * 128: pr * QTOK + (t + 1) * 128],
                            rhs=wo_sb[:, pr * D + nh * 512: pr * D + (nh + 1) * 512],
                            start=(pr == 0), stop=(pr == 3),
                        )
                ob_pend.append((ops, t))
                if len(ob_pend) > 1:
                    ob_flush(*ob_pend.pop(0))
            while ob_pend:
                ob_flush(*ob_pend.pop(0))
            if debug_taps:
                nc.sync.dma_start(dbg["rsin"][:, :], rs_in.ap())
            # exchange halves with the pair partner: AllToAll is a pure
            # byte-move (no CCE arithmetic), so fp16 is safe; each core then
            # sums own+peer partials locally.
            nc.gpsimd.collective_compute(
                "AllGather", OP.bypass,
                replica_groups=[[0, 1], [2, 3], [4, 5], [6, 7]],
                ins=[rs_in.ap().opt()], outs=[rs_out.ap().opt()],
            )
            for c in range(8):
                rmsnorm_chunk(c)
            if debug_taps:
                nc.sync.dma_start(dbg["rsout"][:, :], rs_out.ap())

    nc.compile()
    return nc


TileCtx = tile.TileContext

_PROGRAM = None


def _get_program():
    global _PROGRAM
    if _PROGRAM is None:
        _PROGRAM = build_program()
    return _PROGRAM


def shard_inputs(inputs):
    """Full inputs -> list of 8 per-core in_maps."""
    Q, K, V = (np.asarray(inputs[k], np.float32) for k in ("Q", "K", "V"))
    mask = np.asarray(inputs["mask"])
    wq, wk, wv, wo = (np.asarray(inputs[k], np.float32) for k in ("wq", "wk", "wv", "wo"))
    bq, bk, bv, bo = (np.asarray(inputs[k], np.float32) for k in ("bq", "bk", "bv", "bo"))
    wnq, wnk = (np.asarray(inputs[k], np.float32) for k in ("wnq", "wnk"))
    bnq, bnk = (np.asarray(inputs[k], np.float32) for k in ("bnq", "bnk"))
    temperature = np.asarray(inputs["temperature"], np.float32).reshape(H)
    gamma = np.asarray(inputs["gamma"], np.float32)

    mfull = mask.reshape(S, S).astype(np.float16)
    in_maps = []
    for c in range(N_CORES):
        pair, half = c // 2, c % 2
        b, q0 = pair // 2, (pair % 2) * QTOK
        hs = half * HE
        m = {
            "qt": np.ascontiguousarray(Q[b, q0:q0 + QTOK, :].T),
            "kt": np.ascontiguousarray(K[b].T),
            "vt": np.ascontiguousarray(V[b].T),
            "maskt": np.ascontiguousarray(mfull[q0:q0 + QTOK, :].T),
            "wq": np.ascontiguousarray(wq[:, hs:hs + HE]),
            "wk": np.ascontiguousarray(wk[:, hs:hs + HE]),
            "wv": np.ascontiguousarray(wv[:, hs:hs + HE]),
            "wo": np.ascontiguousarray(wo[hs:hs + HE, :]),
            "bq": np.ascontiguousarray(bq[hs:hs + HE].reshape(4, 128)),
            "bk": np.ascontiguousarray(bk[hs:hs + HE].reshape(4, 128)),
            "bv": np.ascontiguousarray(bv[hs:hs + HE].reshape(4, 128)),
            "bo": np.ascontiguousarray(bo.reshape(1, D)),
            "wnq": np.ascontiguousarray(wnq),
            "wnk": np.ascontiguousarray(wnk),
            "bnq": np.ascontiguousarray(bnq.reshape(DK, 1)),
            "bnk": np.ascontiguousarray(bnk.reshape(DK, 1)),
            "temp": np.ascontiguousarray(temperature[hs // DK: hs // DK + 8].reshape(1, 8)),
            "gamma": np.ascontiguousarray(gamma.reshape(1, D)),
        }
        in_maps.append(m)
    return in_maps


def assemble_output(results):
    out = np.empty((B, S, D), np.float32)
    for c in range(N_CORES):
        pair, half = c // 2, c % 2
        b, q0 = pair // 2, (pair % 2) * QTOK
        out[b, q0 + half * 512: q0 + (half + 1) * 512, :] = \
            results[c]["out"][half * 512:(half + 1) * 512]
    return out


def run(inputs, trace=False, **kwargs):
    nc = _get_program()
    in_maps = shard_inputs(inputs)
    res = run_bass_kernel_spmd(nc, in_maps, list(range(N_CORES)), trace=trace, **kwargs)
    return assemble_output(res.results), res


def kernel(**inputs) -> np.ndarray:
    return run(inputs)[0]


# revision 28
# speedup vs baseline: 1.0312x; 1.0312x over previous
"""AdvancedMuonAttention on 8 TRN2 NeuronCores (Bass/Tile SPMD).

Sharding: 8 cores = 4 pairs. Pair p owns (batch=p//2, query tokens
(p%2)*1024..+1024). Within a pair, core A takes heads 0-7, core B heads
8-15 (tensor-parallel on the head dim: wq/wk/wv column-sliced, wo
row-sliced). Each core computes k/v projections for its batch+heads over
all 2048 keys, attention for its 8 heads x 1024 queries, a partial
output projection, then a 2-rank ReduceScatter(add) with its pair
partner resolves the head-dim partial sums and hands each core 512
tokens for RMSNorm + output.

Math notes vs the reference:
  - softmax is computed without max-subtraction (scores are bounded by
    dk/sqrt(dk)/temp since the feature maps are tanh outputs), with the
    0/1 mask applied multiplicatively AFTER exp (identical result).
  - row sums for the softmax denominator come from an appended
    ones-column on V (ctx' = P @ [V|1]), normalization happens on ctx.
  - bv/bo fold into a constant row c* = bv @ wo + bo added to the
    partial outputs before the ReduceScatter; temperature folds into
    the exp() input scale.

Schedule notes: engines execute their instruction streams in-order, so
feature/ctx matmuls are emitted lagged behind the instructions that
produce their inputs (software pipelining) to keep TensorE dense and
the HAM clock warm.
"""

import os
import sys
import numpy as np
from contextlib import ExitStack

for _p in ("/opt/trn_rl_repo",):
    if _p not in sys.path and os.path.isdir(_p):
        sys.path.append(_p)

import concourse.bass as bass
import concourse.bacc as bacc
import concourse.mybir as mybir
import concourse.tile as tile
from concourse.bass_utils import run_bass_kernel_spmd

F32 = mybir.dt.float32
F32R = mybir.dt.float32r
F16 = mybir.dt.float16
AF = mybir.ActivationFunctionType
OP = mybir.AluOpType

B, S, D, H, DK = 2, 2048, 1024, 16, 64
N_CORES = 8
EPS = 1e-6
QTOK = 1024          # query tokens per pair (and per core)
HE = 512             # head dims per core (8 heads x 64)
CTX_LAG = 4          # attention: ctx matmuls lag scores by this many k-tiles


def build_program(debug_taps=False):
    nc = bacc.Bacc("TRN2", debug=False, num_devices=N_CORES)

    qt = nc.declare_dram_parameter("qt", [D, QTOK], F32R, isOutput=False)
    kt = nc.declare_dram_parameter("kt", [D, S], F32R, isOutput=False)
    vt = nc.declare_dram_parameter("vt", [D, S], F32R, isOutput=False)
    maskt = nc.declare_dram_parameter("maskt", [S, QTOK], F16, isOutput=False)
    wq = nc.declare_dram_parameter("wq", [D, HE], F32R, isOutput=False)
    wk = nc.declare_dram_parameter("wk", [D, HE], F32R, isOutput=False)
    wv = nc.declare_dram_parameter("wv", [D, HE], F32R, isOutput=False)
    wo = nc.declare_dram_parameter("wo", [HE, D], F32R, isOutput=False)
    bq = nc.declare_dram_parameter("bq", [4, 128], F32, isOutput=False)
    bk = nc.declare_dram_parameter("bk", [4, 128], F32, isOutput=False)
    bv = nc.declare_dram_parameter("bv", [4, 128], F32R, isOutput=False)
    bo = nc.declare_dram_parameter("bo", [1, D], F32, isOutput=False)
    wnq = nc.declare_dram_parameter("wnq", [DK, DK], F32R, isOutput=False)
    wnk = nc.declare_dram_parameter("wnk", [DK, DK], F32R, isOutput=False)
    bnq = nc.declare_dram_parameter("bnq", [DK, 1], F32, isOutput=False)
    bnk = nc.declare_dram_parameter("bnk", [DK, 1], F32, isOutput=False)
    temp = nc.declare_dram_parameter("temp", [1, 8], F32, isOutput=False)
    gamma = nc.declare_dram_parameter("gamma", [1, D], F32, isOutput=False)
    out_ext = nc.declare_dram_parameter("out", [QTOK, D], F32, isOutput=True)

    # collective bounce buffers
    rs_in = nc.dram_tensor("rs_in", [QTOK, D], F16)
    rs_out = nc.dram_tensor("rs_out", [2 * QTOK, D], F16)
    dbg = {}
    if debug_taps:
        dbg["kfn"] = nc.declare_dram_parameter("dbg_kfn", [128, 4 * S], F16, isOutput=True)
        dbg["qfn"] = nc.declare_dram_parameter("dbg_qfn", [128, 4 * QTOK], F16, isOutput=True)
        dbg["vsb"] = nc.declare_dram_parameter("dbg_vsb", [128, 16 * 520], F16, isOutput=True)
        dbg["ctx"] = nc.declare_dram_parameter("dbg_ctx", [128, 4 * QTOK], F32, isOutput=True)
        dbg["p00"] = nc.declare_dram_parameter("dbg_p00", [128, QTOK], F16, isOutput=True)
        dbg["rsin"] = nc.declare_dram_parameter("dbg_rsin", [QTOK, D], F32, isOutput=True)
        dbg["rsout"] = nc.declare_dram_parameter("dbg_rsout", [512, D], F32, isOutput=True)

    with TileCtx(nc) as tc, ExitStack() as top:
        persist = top.enter_context(tc.tile_pool(name="persist", bufs=1))

        # ---- persistent tiles -------------------------------------------
        mask_sb = persist.tile([128, 16 * QTOK], F16, tag="mask")
        kfn_sb = persist.tile([128, 4 * S], F16, tag="kfn")      # [he-pair rows, pair-block of 2048 tok]
        qfn_sb = persist.tile([128, 4 * QTOK], F16, tag="qfn")
        v_sb = persist.tile([128, 16 * 520], F16, tag="vsb")     # per tok-tile: 8 heads x (64 v + 1 one)
        ctx_sb = persist.tile([128, 4 * QTOK], F32R, tag="ctx")
        cstar = persist.tile([1, D], F32, tag="cstar")
        gamma_sb = persist.tile([1, D], F32, tag="gamma")
        bo_sb = persist.tile([1, D], F32, tag="bo")
        bq_sb = persist.tile([128, 4], F32, tag="bq")
        bk_sb = persist.tile([128, 4], F32, tag="bk")
        bv_sb = persist.tile([128, 4], F32R, tag="bv")
        wnq_d = persist.tile([128, 2 * DK], F32R, tag="wnq")   # block-diag [[wn,0],[0,wn]]
        wnk_d = persist.tile([128, 2 * DK], F32R, tag="wnk")
        bnq_d = persist.tile([128, 1], F32, tag="bnq")
        bnk_d = persist.tile([128, 1], F32, tag="bnk")
        eps_t = persist.tile([128, 1], F32, tag="eps")
        ts_raw = persist.tile([1, 8], F32, tag="tsraw")
        ts_rec = persist.tile([1, 8], F32, tag="tsrec")
        s_bc = persist.tile([128, 8], F32, tag="sbc")
        gamma_bc = persist.tile([128, D], F32, tag="gammabc")
        cstar_bc = persist.tile([128, D], F32, tag="cstarbc")

        # ---- constants / small DMAs -------------------------------------
        nc.vector.memset(eps_t[:], EPS)
        nc.sync.dma_start(gamma_sb[:], gamma[:, :])
        nc.sync.dma_start(bo_sb[:], bo[:, :])
        nc.sync.dma_start(bq_sb[:], bq.ap().rearrange("t p -> p t"))
        nc.sync.dma_start(bk_sb[:], bk.ap().rearrange("t p -> p t"))
        nc.sync.dma_start(bv_sb[:], bv.ap().rearrange("t p -> p t"))
        nc.vector.memset(wnq_d[:].bitcast(F32), 0.0)
        nc.vector.memset(wnk_d[:].bitcast(F32), 0.0)
        nc.sync.dma_start(wnq_d[0:64, 0:64], wnq[:, :])
        nc.sync.dma_start(wnq_d[64:128, 64:128], wnq[:, :])
        nc.sync.dma_start(wnk_d[0:64, 0:64], wnk[:, :])
        nc.sync.dma_start(wnk_d[64:128, 64:128], wnk[:, :])
        nc.sync.dma_start(bnq_d[0:64, :], bnq[:, :])
        nc.sync.dma_start(bnq_d[64:128, :], bnq[:, :])
        nc.sync.dma_start(bnk_d[0:64, :], bnk[:, :])
        nc.sync.dma_start(bnk_d[64:128, :], bnk[:, :])
        nc.sync.dma_start(ts_raw[:], temp[:, :])
        # exp input scale per head: 1/(sqrt(DK) * temperature_h)
        nc.vector.reciprocal(ts_rec[:], ts_raw[:])
        nc.vector.tensor_scalar_mul(ts_rec[:], ts_rec[:], 1.0 / np.sqrt(DK))
        nc.gpsimd.partition_broadcast(s_bc[:], ts_rec[0:1, :])
        nc.gpsimd.partition_broadcast(gamma_bc[:], gamma_sb[0:1, :])

        # ones columns for the V|1 trick
        nc.gpsimd.memset(v_sb[:], 1.0)

        # ==== Phase 1: projections + feature maps ========================
        with ExitStack() as ph1:
            wpool = ph1.enter_context(tc.tile_pool(name="wpool", bufs=1))
            inpool = ph1.enter_context(tc.tile_pool(name="inpool", bufs=10))
            vtpool = ph1.enter_context(tc.tile_pool(name="vtpool", bufs=8))
            kqpool = ph1.enter_context(tc.tile_pool(name="kqpool", bufs=6))
            ppsum = ph1.enter_context(tc.tile_pool(name="ppsum", bufs=3, space="PSUM"))
            fpsum = ph1.enter_context(tc.tile_pool(name="fpsum", bufs=2, space="PSUM"))
            wq_sb = wpool.tile([128, 8 * HE], F32R, tag="wq")
            wk_sb = wpool.tile([128, 8 * HE], F32R, tag="wk")
            wv_sb = wpool.tile([128, 8 * HE], F32R, tag="wv")
            # interleave the first k-stream chunk with wk so the first
            # projection chain starts as early as possible
            kt_j0 = []
            for dt_ in range(8):
                nc.sync.dma_start(wk_sb[:, dt_ * HE:(dt_ + 1) * HE], wk[dt_ * 128:(dt_ + 1) * 128, :])
                t = inpool.tile([128, 512], F32R, tag="instream", name=f"kt0_{dt_}")
                nc.scalar.dma_start(t[:], kt[dt_ * 128:(dt_ + 1) * 128, 0:512])
                kt_j0.append(t)


            # software pipeline: the feature matmul for step i is emitted
            # during step i+1, so TensorE never waits on the DVE bias-add.
            feat_pend = []

            def feat_flush_one():
                xsb, wn_d, bn_d, dst = feat_pend.pop(0)
                fps = fpsum.tile([128, 512], F32, tag="feat")
                nc.tensor.matmul(fps[:], lhsT=wn_d[:], rhs=xsb[:],
                                 start=True, stop=True)
                nc.scalar.activation(dst, fps[:], AF.Tanh, bias=bn_d[:, 0:1])

            def proj_feat(src_dram, w_sb, bias_sb, wn_d, bn_d, fn_sb, n_tok, pre0=None):
                """project src (transposed [D, n_tok]) through w (column
                slice) then per-head feature map tanh((x+b) @ wn + bn),
                writing fp16 features [he, n_tok] into fn_sb."""
                for j in range(n_tok // 512):
                    if j == 0 and pre0 is not None:
                        ins = pre0
                    else:
                        ins = []
                        for dt_ in range(8):
                            t = inpool.tile([128, 512], F32R, tag="instream")
                            nc.scalar.dma_start(t[:], src_dram[dt_ * 128:(dt_ + 1) * 128, j * 512:(j + 1) * 512])
                            ins.append(t)
                    for i in range(4):
                        ps = ppsum.tile([128, 512], F32, tag="proj")
                        for dt_ in range(8):
                            nc.tensor.matmul(
                                ps[:],
                                lhsT=w_sb[:, dt_ * HE + i * 128: dt_ * HE + (i + 1) * 128],
                                rhs=ins[dt_][:],
                                start=(dt_ == 0), stop=(dt_ == 7),
                            )
                        xsb = kqpool.tile([128, 512], F32R, tag="xsb")
                        nc.vector.tensor_scalar_add(xsb[:], ps[:], bias_sb[:, i:i + 1])
                        feat_pend.append((
                            xsb, wn_d, bn_d,
                            fn_sb[:, i * n_tok + j * 512: i * n_tok + (j + 1) * 512],
                        ))
                        if len(feat_pend) > 1:
                            feat_flush_one()

            for dt_ in range(8):
                nc.sync.dma_start(wq_sb[:, dt_ * HE:(dt_ + 1) * HE], wq[dt_ * 128:(dt_ + 1) * 128, :])
            proj_feat(kt, wk_sb, bk_sb, wnk_d, bnk_d, kfn_sb, S, pre0=kt_j0)
            for dt_ in range(8):
                nc.sync.dma_start(wv_sb[:, dt_ * HE:(dt_ + 1) * HE], wv[dt_ * 128:(dt_ + 1) * 128, :])
            proj_feat(qt, wq_sb, bq_sb, wnq_d, bnq_d, qfn_sb, QTOK)
            while feat_pend:
                feat_flush_one()
            # fold the per-head softmax scale 1/(sqrt(dk)*temp_h) into qfn
            for h in range(8):
                pr, off = h // 2, (h % 2) * 64
                sl = qfn_sb[off:off + 64, pr * QTOK:(pr + 1) * QTOK]
                nc.vector.tensor_scalar_mul(sl, sl, s_bc[0:64, h:h + 1])

            # v projection: [tok, he] via VT-tiles as stationary.
            # The v_sb copy lags one chain so TensorE stays dense.
            def v_copy(vps0, t0):
                dst = v_sb[:, t0 * 520:(t0 + 1) * 520].rearrange("p (h c) -> p h c", c=65)[:, :, 0:64]
                nc.vector.tensor_copy(dst, vps0[:].rearrange("p (h c) -> p h c", c=64))

            v_pend = []
            for tc4 in range(4):
                vts = []
                for dt_ in range(8):
                    vtt = vtpool.tile([128, 512], F32R, tag="vtstream")
                    nc.scalar.dma_start(vtt[:], vt[dt_ * 128:(dt_ + 1) * 128, tc4 * 512:(tc4 + 1) * 512])
                    vts.append(vtt)
                for sub in range(4):
                    t = tc4 * 4 + sub
                    vps = ppsum.tile([128, 512], F32, tag="proj")
                    for dt_ in range(8):
                        nc.tensor.matmul(
                            vps[:], lhsT=vts[dt_][:, sub * 128:(sub + 1) * 128],
                            rhs=wv_sb[:, dt_ * HE:(dt_ + 1) * HE],
                            start=(dt_ == 0), stop=(dt_ == 7),
                        )
                    v_pend.append((vps, t))
                    if len(v_pend) > 1:
                        v_copy(*v_pend.pop(0))
            while v_pend:
                v_copy(*v_pend.pop(0))

        # wo is only consumed in phase 3, but its DMA is issued here so it
        # overlaps the attention phase
        wopool = top.enter_context(tc.tile_pool(name="wopool", bufs=1))
        wo_sb = wopool.tile([128, 4 * D], F32R, tag="wo")
        for pr in range(4):
            nc.sync.dma_start(
                wo_sb[:, pr * D:(pr + 1) * D], wo[pr * 128:(pr + 1) * 128, :]
            )

        # ==== Phase 2: attention =========================================
        with ExitStack() as ph2:
            spsum = ph2.enter_context(tc.tile_pool(name="spsum", bufs=2, space="PSUM"))
            cpsum = ph2.enter_context(tc.tile_pool(name="cpsum", bufs=2, space="PSUM"))
            ptpool = ph2.enter_context(tc.tile_pool(name="ptpool", bufs=10))
            rpool = ph2.enter_context(tc.tile_pool(name="rpool", bufs=2))

            # mask tiles: k-tile kti lives at cols kti*QTOK (deferred DMA so
            # the startup path belongs to the projection streams)
            for kti in range(16):
                nc.sync.dma_start(
                    mask_sb[:, kti * QTOK:(kti + 1) * QTOK],
                    maskt[kti * 128:(kti + 1) * 128, :],
                )

            # One flat software pipeline over (head, k-tile): the ctx
            # accumulation lags the scores stream by CTX_LAG steps and the
            # per-head normalize is emitted inline, so TensorE crosses head
            # boundaries without draining.
            steps = [(h, kti) for h in range(8) for kti in range(16)]
            cps_by_head = {}
            pts = {}

            def ctx_step(h, kti):
                cps2 = cps_by_head[h]
                pt = pts.pop((h, kti))
                for qh in range(2):
                    nc.tensor.matmul(
                        cps2[:, qh * 512:(qh + 1) * 512],
                        lhsT=v_sb[:, kti * 520 + h * 65: kti * 520 + (h + 1) * 65],
                        rhs=pt[:, qh * 512:(qh + 1) * 512],
                        start=(kti == 0), stop=(kti == 15),
                    )
                if kti == 15:
                    pr, off = h // 2, (h % 2) * 64
                    rec = rpool.tile([1, QTOK], F32, tag="rec")
                    nc.vector.reciprocal(rec[:], cps2[64:65, :])
                    rec_bc = rpool.tile([64, QTOK], F32, tag="recbc")
                    nc.gpsimd.partition_broadcast(rec_bc[:], rec[0:1, :])
                    nc.vector.tensor_tensor(
                        ctx_sb[off:off + 64, pr * QTOK:(pr + 1) * QTOK],
                        cps2[0:64, :], rec_bc[:], OP.mult,
                    )
                    del cps_by_head[h]

            for idx, (h, kti) in enumerate(steps):
                pr, off = h // 2, (h % 2) * 64
                if kti == 0:
                    cps_by_head[h] = cpsum.tile([65, QTOK], F32, tag="ctxps", name=f"ctxps{h}")
                sps = spsum.tile([128, QTOK], F32, tag="scores")
                for qh in range(2):
                    nc.tensor.matmul(
                        sps[:, qh * 512:(qh + 1) * 512],
                        lhsT=kfn_sb[off:off + 64, pr * S + kti * 128: pr * S + (kti + 1) * 128],
                        rhs=qfn_sb[off:off + 64, pr * QTOK + qh * 512: pr * QTOK + (qh + 1) * 512],
                        start=True, stop=True,
                    )
                pt = ptpool.tile([128, QTOK], F16, tag="pt")
                nc.scalar.activation(pt[:], sps[:], AF.Exp)
                nc.vector.tensor_mul(pt[:], pt[:], mask_sb[:, kti * QTOK:(kti + 1) * QTOK])
                if debug_taps and h == 0 and kti == 0:
                    nc.sync.dma_start(dbg["p00"][:, :], pt[:])
                pts[(h, kti)] = pt
                if idx >= CTX_LAG:
                    ctx_step(*steps[idx - CTX_LAG])
            for idx in range(len(steps) - CTX_LAG, len(steps)):
                ctx_step(*steps[idx])

        if debug_taps:
            nc.sync.dma_start(dbg["kfn"][:, :], kfn_sb[:])
            nc.sync.dma_start(dbg["qfn"][:, :], qfn_sb[:])
            nc.sync.dma_start(dbg["vsb"][:, :], v_sb[:])
            nc.sync.dma_start(dbg["ctx"][:, :], ctx_sb[:].bitcast(F32))

        # ==== Phase 3: output projection + chunked ReduceScatter =========
        # Two RS chunks: chunk c covers pair-token rows c*512..(c+1)*512.
        # The even core ends up with rows [c*512, c*512+256) of each chunk.
        with ExitStack() as ph3:
            opsum = ph3.enter_context(tc.tile_pool(name="opsum", bufs=2, space="PSUM"))
            obpool = ph3.enter_context(tc.tile_pool(name="obpool", bufs=3))
            fpool = ph3.enter_context(tc.tile_pool(name="fpool", bufs=3))
            sqpool = ph3.enter_context(tc.tile_pool(name="sqpool", bufs=2))
            # c* = bv @ wo + 0.5*bo   (both pair cores add 0.5*bo)
            cps = opsum.tile([1, D], F32, tag="cstar_ps")
            for pr in range(4):
                for nh in range(2):
                    nc.tensor.matmul(
                        cps[:, nh * 512:(nh + 1) * 512],
                        lhsT=bv_sb[:, pr:pr + 1],
                        rhs=wo_sb[:, pr * D + nh * 512: pr * D + (nh + 1) * 512],
                        start=(pr == 0), stop=(pr == 3),
                    )
            nc.vector.tensor_scalar_mul(cstar[:], bo_sb[:], 0.5)
            nc.vector.tensor_add(cstar[:], cstar[:], cps[:])
            nc.gpsimd.partition_broadcast(cstar_bc[:], cstar[0:1, :])

            def ob_flush(ops0, t0):
                ob = obpool.tile([128, D], F16, tag="ob")
                nc.vector.tensor_tensor(ob[:], ops0[:], cstar_bc[:], OP.add)
                nc.sync.dma_start(rs_in[t0 * 128:(t0 + 1) * 128, :], ob[:])

            def rmsnorm_chunk(c):
                # ag out: rows [0:1024) = rank-0 partials, [1024:2048) =
                # rank-1 partials, both in pair-token order. Each core norms
                # all 1024 pair tokens (SPMD-uniform); the host keeps the
                # 512 rows this core owns.
                if True:
                    r0 = c * 128
                    oa = fpool.tile([128, D], F16, tag="oa")
                    nc.sync.dma_start(oa[:], rs_out[r0:r0 + 128, :])
                    obp = fpool.tile([128, D], F16, tag="obp")
                    nc.sync.dma_start(obp[:], rs_out[QTOK + r0:QTOK + r0 + 128, :])
                    o = fpool.tile([128, D], F32, tag="o")
                    nc.vector.tensor_add(o[:], oa[:], obp[:])
                    sq = sqpool.tile([128, D], F16, tag="sq")
                    ss = sqpool.tile([128, 1], F32, tag="ss")
                    nc.scalar.activation(sq[:], o[:], AF.Square, accum_out=ss[:])
                    rms = sqpool.tile([128, 1], F32, tag="rms")
                    nc.scalar.activation(rms[:], ss[:], AF.Sqrt, bias=eps_t[:, 0:1], scale=1.0 / D)
                    rinv = sqpool.tile([128, 1], F32, tag="rinv")
                    nc.vector.reciprocal(rinv[:], rms[:])
                    o2 = fpool.tile([128, D], F32, tag="o2")
                    nc.vector.tensor_scalar_mul(o2[:], o[:], rinv[:, 0:1])
                    nc.gpsimd.tensor_tensor(o2[:], o2[:], gamma_bc[:], OP.mult)
                    nc.sync.dma_start(out_ext[r0:r0 + 128, :], o2[:])

            ob_pend = []
            for t in range(8):
                ops = opsum.tile([128, D], F32, tag="ops")
                for pr in range(4):
                    for nh in range(2):
                        nc.tensor.matmul(
                            ops[:, nh * 512:(nh + 1) * 512],
                            lhsT=ctx_sb[:, pr * QTOK + t * 128: pr * QTOK + (t + 1) * 128],
                            rhs=wo_sb[:, pr * D + nh * 512: pr * D + (nh + 1) * 512],
                            start=(pr == 0), stop=(pr == 3),
                        )
                ob_pend.append((ops, t))
                if len(ob_pend) > 1:
                    ob_flush(*ob_pend.pop(0))
            while ob_pend:
                ob_flush(*ob_pend.pop(0))
            if debug_taps:
                nc.sync.dma_start(dbg["rsin"][:, :], rs_in.ap())
            # exchange halves with the pair partner: AllToAll is a pure
            # byte-move (no CCE arithmetic), so fp16 is safe; each core then
            # sums own+peer partials locally.
            nc.gpsimd.collective_compute(
                "AllGather", OP.bypass,
                replica_groups=[[0, 1], [2, 3], [4, 5], [6, 7]],
                ins=[rs_in.ap().opt()], outs=[rs_out.ap().opt()],
            )
            for c in range(8):
                rmsnorm_chunk(c)
            if debug_taps:
                nc.sync.dma_start(dbg["rsout"][:, :], rs_out.ap())

    nc.compile()
    return nc


TileCtx = tile.TileContext

_PROGRAM = None


def _get_program():
    global _PROGRAM
    if _PROGRAM is None:
        _PROGRAM = build_program()
    return _PROGRAM


def shard_inputs(inputs):
    """Full inputs -> list of 8 per-core in_maps."""
    Q, K, V = (np.asarray(inputs[k], np.float32) for k in ("Q", "K", "V"))
    mask = np.asarray(inputs["mask"])
    wq, wk, wv, wo = (np.asarray(inputs[k], np.float32) for k in ("wq", "wk", "wv", "wo"))
    bq, bk, bv, bo = (np.asarray(inputs[k], np.float32) for k in ("bq", "bk", "bv", "bo"))
    wnq, wnk = (np.asarray(inputs[k], np.float32) for k in ("wnq", "wnk"))
    bnq, bnk = (np.asarray(inputs[k], np.float32) for k in ("bnq", "bnk"))
    temperature = np.asarray(inputs["temperature"], np.float32).reshape(H)
    gamma = np.asarray(inputs["gamma"], np.float32)

    mfull = mask.reshape(S, S).astype(np.float16)
    in_maps = []
    for c in range(N_CORES):
        pair, half = c // 2, c % 2
        b, q0 = pair // 2, (pair % 2) * QTOK
        hs = half * HE
        m = {
            "qt": np.ascontiguousarray(Q[b, q0:q0 + QTOK, :].T),
            "kt": np.ascontiguousarray(K[b].T),
            "vt": np.ascontiguousarray(V[b].T),
            "maskt": np.ascontiguousarray(mfull[q0:q0 + QTOK, :].T),
            "wq": np.ascontiguousarray(wq[:, hs:hs + HE]),
            "wk": np.ascontiguousarray(wk[:, hs:hs + HE]),
            "wv": np.ascontiguousarray(wv[:, hs:hs + HE]),
            "wo": np.ascontiguousarray(wo[hs:hs + HE, :]),
            "bq": np.ascontiguousarray(bq[hs:hs + HE].reshape(4, 128)),
            "bk": np.ascontiguousarray(bk[hs:hs + HE].reshape(4, 128)),
            "bv": np.ascontiguousarray(bv[hs:hs + HE].reshape(4, 128)),
            "bo": np.ascontiguousarray(bo.reshape(1, D)),
            "wnq": np.ascontiguousarray(wnq),
            "wnk": np.ascontiguousarray(wnk),
            "bnq": np.ascontiguousarray(bnq.reshape(DK, 1)),
            "bnk": np.ascontiguousarray(bnk.reshape(DK, 1)),
            "temp": np.ascontiguousarray(temperature[hs // DK: hs // DK + 8].reshape(1, 8)),
            "gamma": np.ascontiguousarray(gamma.reshape(1, D)),
        }
        in_maps.append(m)
    return in_maps


def assemble_output(results):
    out = np.empty((B, S, D), np.float32)
    for c in range(N_CORES):
        pair, half = c // 2, c % 2
        b, q0 = pair // 2, (pair % 2) * QTOK
        out[b, q0 + half * 512: q0 + (half + 1) * 512, :] = \
            results[c]["out"][half * 512:(half + 1) * 512]
    return out


def run(inputs, trace=False, **kwargs):
    nc = _get_program()
    in_maps = shard_inputs(inputs)
    res = run_bass_kernel_spmd(nc, in_maps, list(range(N_CORES)), trace=trace, **kwargs)
    return assemble_output(res.results), res


def kernel(**inputs) -> np.ndarray:
    return run(inputs)[0]
